# revision 1
# baseline (speedup 1.0000x reference)
"""DAGNConv (GNN message passing) Trainium2 kernel.

Strategy (8 NeuronCores, SPMD):
  - Sort edges by head (destination) node on host; shard nodes contiguously
    12500/core so each core owns the edges whose heads fall in its shard.
    Segment softmax is then core-local.
  - Score phase: per-NODE score tables s_h/s_t/s_r (N-sized work, not
    E-sized).  Per-edge assembly:
      s_t rides the iteration-1 tail gather (combined [ent|s_t] table),
      s_h via PE-transpose of the on-chip one-hot chunk (matmul),
      s_r via a host-shipped relation one-hot (bf16) matmul against the
      on-device relation table.
    Attention denominators are folded into a per-node scale (no per-edge
    division); exp/leaky-relu computed per chunk on chip.
  - Power iterations: per 128-edge chunk, indirect-DMA row gather of
    Z[tails] (HW semantics: one row per partition per instruction), then
    segment-sum via one-hot matmul: S^T chunks built ON DEVICE with an
    is_equal broadcast compare against an iota row.  PSUM accumulates
    [128 nodes, 256] over a node tile's chunks.
  - Between iterations: AllGather of the [12544, 256] Z shards.
  - Output: Z5 @ W_o folded into iteration 5 (PE transpose + matmul).
"""

import os
import sys

import numpy as np

for _p in ("/opt/trn_rl_repo",):
    if _p not in sys.path and os.path.isdir(_p):
        sys.path.insert(0, _p)

P = 128
N_ENT = 100000
N_EDGE = 500000
N_REL = 200
DIM = 64
HEADS = 4
HD = HEADS * DIM  # 256
CROW = DIM + HEADS  # combined [ent | s_t] row length (68)
POW_ITER = 5
ALPHA = 0.1
LEAKY = 0.01
EPS = 1e-16
NCORES = 8


class Cfg:
    debug = False

    def __init__(self, n_cores, n_nodes, dim, heads, n_rel, pow_iter, l_cap,
                 alpha=ALPHA, leaky=LEAKY, eps=EPS):
        assert n_nodes % n_cores == 0
        self.n_cores = n_cores
        self.dim = dim
        self.heads = heads
        self.hd = heads * dim
        self.crow = dim + heads
        self.n_rel = n_rel
        self.pow_iter = pow_iter
        self.alpha = alpha
        self.leaky = leaky
        self.eps = eps
        self.npc = n_nodes // n_cores          # real nodes per core
        self.nt = -(-self.npc // P)            # node tiles per core
        self.nps = self.nt * P                 # padded nodes per core
        self.npt = self.nps * n_cores          # padded total nodes
        assert l_cap % P == 0
        self.l_cap = l_cap
        self.ch = l_cap // P                   # chunks per node tile
        self.nchunk = self.nt * self.ch        # chunks per core
        self.rp = -(-n_rel // P) * P           # padded relations


def host_prep(cfg, entity_embed, relation_embed, edge_index, edge_type):
    """Sort/shard edges, build per-core slot arrays. Returns per-core dicts."""
    import ml_dtypes

    h = np.asarray(edge_index[0], dtype=np.int64)
    t = np.asarray(edge_index[1], dtype=np.int64)
    r = np.asarray(edge_type, dtype=np.int64)
    perm = np.argsort(h, kind="stable")
    hs, ts, rs = h[perm], t[perm], r[perm]

    def pad_idx(x):
        return ((x // cfg.npc) * cfg.nps + (x % cfg.npc)).astype(np.int32)

    tile_edges = []
    max_cnt = 1
    for c in range(cfg.n_cores):
        gbase = c * cfg.npc
        lo = np.searchsorted(hs, gbase)
        hi = np.searchsorted(hs, gbase + cfg.npc)
        tb = np.searchsorted(hs[lo:hi], gbase + np.arange(0, cfg.nt + 1) * P) + lo
        tile_edges.append((gbase, tb))
        max_cnt = max(max_cnt, int(np.max(tb[1:] - tb[:-1])))
    l_cap = -(-max_cnt // P) * P
    if l_cap != cfg.l_cap:
        ncfg = Cfg(cfg.n_cores, cfg.npc * cfg.n_cores, cfg.dim, cfg.heads,
                   cfg.n_rel, cfg.pow_iter, l_cap,
                   cfg.alpha, cfg.leaky, cfg.eps)
        ncfg.debug = cfg.debug
        cfg = ncfg

    ent = np.asarray(entity_embed, dtype=np.float32)
    rel = np.asarray(relation_embed, dtype=np.float32)
    shards = []
    for c in range(cfg.n_cores):
        sh = np.zeros((cfg.nps, cfg.dim), np.float32)
        sh[:cfg.npc] = ent[c * cfg.npc:(c + 1) * cfg.npc]
        shards.append(sh)
    ent68 = np.zeros((cfg.npt, cfg.crow), np.float32)
    ent68[:, :cfg.dim] = np.concatenate(shards, axis=0)
    rel_pad = np.zeros((cfg.rp, cfg.dim), np.float32)
    rel_pad[:cfg.n_rel] = rel

    cores = []
    for c in range(cfg.n_cores):
        gbase, tb = tile_edges[c]
        h_rel = np.full((P, cfg.nchunk), -1.0, np.float32)
        tti = np.zeros((P, cfg.nchunk), np.int32)
        r1h = np.zeros((cfg.rp, cfg.nchunk * P), ml_dtypes.bfloat16)
        for i in range(cfg.nt):
            lo, hi = int(tb[i]), int(tb[i + 1])
            cnt = hi - lo
            if cnt == 0:
                continue
            j = np.arange(cnt)
            cols = i * cfg.ch + (j // P)
            parts = j % P
            h_rel[parts, cols] = (hs[lo:hi] - (gbase + i * P)).astype(np.float32)
            tti[parts, cols] = pad_idx(ts[lo:hi])
            r1h[rs[lo:hi], cols * P + parts] = 1.0
        cores.append(dict(h_rel=h_rel, tti=tti, r1h=r1h,
                          ent_shard=shards[c]))
    return cfg, cores, ent68, rel_pad


def build_program(cfg):
    import concourse.bass as bass
    import concourse.bacc as bacc
    import concourse.mybir as mybir
    from concourse.masks import make_identity
    from concourse.tile import TileContext

    f32 = mybir.dt.float32
    i32 = mybir.dt.int32
    bf16 = mybir.dt.bfloat16
    AF = mybir.ActivationFunctionType
    OP = mybir.AluOpType
    AX = mybir.AxisListType
    H, D, HDc = cfg.heads, cfg.dim, cfg.hd
    CR = cfg.crow
    CH, NT = cfg.ch, cfg.nt
    RB = cfg.rp // P  # relation K-blocks

    nc = bacc.Bacc("TRN2", target_bir_lowering=False, debug=False,
                   num_devices=cfg.n_cores)

    # ---- I/O ----
    ent68_d = nc.dram_tensor("ent68", [cfg.npt, CR], f32, kind="ExternalInput")
    ent_shard_d = nc.dram_tensor("ent_shard", [cfg.nps, D], f32, kind="ExternalInput")
    rel_pad_d = nc.dram_tensor("rel_pad", [cfg.rp, D], f32, kind="ExternalInput")
    wh_d = nc.dram_tensor("wh", [D, HDc], f32, kind="ExternalInput")
    wt_d = nc.dram_tensor("wt", [D, HDc], f32, kind="ExternalInput")
    wr_d = nc.dram_tensor("wr", [D, HDc], f32, kind="ExternalInput")
    wo_d = nc.dram_tensor("wo", [HDc, D], f32, kind="ExternalInput")
    atth_d = nc.dram_tensor("atth", [P, HDc], f32, kind="ExternalInput")
    attt_d = nc.dram_tensor("attt", [P, HDc], f32, kind="ExternalInput")
    attr_d = nc.dram_tensor("attr", [P, HDc], f32, kind="ExternalInput")
    hrel_d = nc.dram_tensor("h_rel", [P, cfg.nchunk], f32, kind="ExternalInput")
    tti_d = nc.dram_tensor("tti", [P, cfg.nchunk], i32, kind="ExternalInput")
    r1h_d = nc.dram_tensor("r1h", [cfg.rp, cfg.nchunk * P], bf16,
                           kind="ExternalInput")
    out_d = nc.dram_tensor("out", [cfg.nps, D], f32, kind="ExternalOutput")

    # ---- internal DRAM ----
    shared = "Shared" if cfg.n_cores > 4 else "Local"
    z_shard = nc.dram_tensor("z_shard", [cfg.nps, HDc], f32)
    z_full = [nc.dram_tensor(f"z_full{i}", [cfg.npt, HDc], f32,
                             addr_space=shared) for i in range(2)]
    st_shard = nc.dram_tensor("st_shard", [cfg.nps, H], f32)
    st_full = nc.dram_tensor("st_full", [cfg.npt, H], f32, addr_space=shared)
    comb = nc.dram_tensor("comb", [cfg.npt, CR], f32)

    rg = [list(range(cfg.n_cores))]

    if cfg.debug:
        dbg_w = nc.dram_tensor("dbg_w", [P, cfg.nchunk * H], f32,
                               kind="ExternalOutput")
        dbg_st = nc.dram_tensor("dbg_st", [cfg.npt, H], f32,
                                kind="ExternalOutput")
        dbg_sh = nc.dram_tensor("dbg_sh", [P, NT * H], f32,
                                kind="ExternalOutput")
        dbg_z1 = nc.dram_tensor("dbg_z1", [cfg.npt, HDc], f32,
                                kind="ExternalOutput")
        dbg_zg = nc.dram_tensor("dbg_zg", [P, CH * CR], f32,
                                kind="ExternalOutput")
        dbg_inv = nc.dram_tensor("dbg_inv", [P, NT * H], f32,
                                 kind="ExternalOutput")

    with TileContext(nc) as tc:
        with (
            tc.tile_pool(name="const", bufs=1) as cp,
            tc.tile_pool(name="work", bufs=3) as wk,
            tc.tile_pool(name="small", bufs=4) as sm,
            tc.tile_pool(name="zg", bufs=3) as zgp,
            tc.tile_pool(name="msg", bufs=3) as msgp,
            tc.tile_pool(name="sone", bufs=3) as sop,
            tc.tile_pool(name="r1", bufs=3) as r1p,
            tc.tile_pool(name="ppA", bufs=2, space="PSUM") as ppA,
            tc.tile_pool(name="ppB", bufs=2, space="PSUM") as ppB,
            tc.tile_pool(name="ppC", bufs=2, space="PSUM") as ppC,
            tc.tile_pool(name="ppD", bufs=1, space="PSUM") as ppD,
        ):
            # ---- constants ----
            ident = cp.tile([P, P], f32, tag="ident")
            make_identity(nc, ident[:])
            iota_i = cp.tile([P, P], i32, tag="iota_i")
            nc.gpsimd.iota(iota_i[:], pattern=[[1, P]], base=0,
                           channel_multiplier=0)
            iota_f = cp.tile([P, P], f32, tag="iota_f")
            nc.vector.tensor_copy(iota_f[:], iota_i[:])

            def load_const(dram, shape, tag):
                t = cp.tile(shape, f32, tag=tag)
                nc.sync.dma_start(t[:], dram[:, :])
                return t

            wh_t = load_const(wh_d, [D, HDc], "wh")
            wt_t = load_const(wt_d, [D, HDc], "wt")
            wr_t = load_const(wr_d, [D, HDc], "wr")
            atth_t = load_const(atth_d, [P, HDc], "atth")
            attt_t = load_const(attt_d, [P, HDc], "attt")
            attr_t = load_const(attr_d, [P, HDc], "attr")
            wo_t = cp.tile([P, 2 * D], f32, tag="wo")
            nc.sync.dma_start(wo_t[:, :D], wo_d[0:P, :])
            nc.sync.dma_start(wo_t[:, D:], wo_d[P:HDc, :])

            hrel_t = cp.tile([P, cfg.nchunk], f32, tag="hrel")
            nc.sync.dma_start(hrel_t[:], hrel_d[:, :])
            tti_t = cp.tile([P, cfg.nchunk], i32, tag="tti")
            nc.sync.dma_start(tti_t[:], tti_d[:, :])

            w_sb = cp.tile([P, cfg.nchunk * H], f32, tag="w")
            inv_sb = cp.tile([P, NT * H], f32, tag="inv")
            sh_all = cp.tile([P, NT * H], f32, tag="sh_all")
            sr_f = cp.tile([P, RB * H], f32, tag="sr_f")
            sr_b = cp.tile([P, RB * H], bf16, tag="sr_b")

            # ---- phase 1: node/relation score tables ----
            def table_pass(src_d, n_tiles, targets):
                # targets: list of (W_tile, att_tile, sink(i, s4_ap))
                for i in range(n_tiles):
                    ent = wk.tile([P, D], f32, tag="ent")
                    nc.sync.dma_start(ent[:], src_d[i * P:(i + 1) * P, :])
                    tp = ppB.tile([P, P], f32, tag="tp")
                    nc.tensor.transpose(out=tp[:D, :], in_=ent[:, :],
                                        identity=ident[:])
                    entT = wk.tile([P, P], f32, tag="entT")
                    nc.vector.tensor_copy(entT[:D, :], tp[:D, :])
                    for (W_t, att_t, sink) in targets:
                        pj = ppA.tile([P, HDc], f32, tag="mm")
                        nc.tensor.matmul(pj[:, :], lhsT=entT[:D, :],
                                         rhs=W_t[:, :], start=True, stop=True)
                        th_ = wk.tile([P, HDc], f32, tag="tanh")
                        nc.scalar.activation(th_[:], pj[:, :], AF.Tanh)
                        pr = wk.tile([P, HDc], f32, tag="prod")
                        nc.vector.tensor_tensor(out=pr[:], in0=th_[:],
                                                in1=att_t[:], op=OP.mult)
                        sink(i, pr)

            def reduce_to(dst_ap):
                def sink(i, pr):
                    nc.vector.tensor_reduce(
                        out=dst_ap(i),
                        in_=pr[:].rearrange("p (h d) -> p h d", h=H),
                        axis=AX.X, op=OP.add)
                return sink

            def st_sink(i, pr):
                s4 = sm.tile([P, H], f32, tag="s4")
                nc.vector.tensor_reduce(
                    out=s4[:, :],
                    in_=pr[:].rearrange("p (h d) -> p h d", h=H),
                    axis=AX.X, op=OP.add)
                nc.sync.dma_start(st_shard[i * P:(i + 1) * P, :], s4[:])

            table_pass(ent_shard_d, NT, [
                (wh_t, atth_t, reduce_to(lambda i: sh_all[:, i * H:(i + 1) * H])),
                (wt_t, attt_t, st_sink),
            ])
            table_pass(rel_pad_d, RB, [
                (wr_t, attr_t, reduce_to(lambda i: sr_f[:, i * H:(i + 1) * H])),
            ])
            nc.vector.tensor_copy(sr_b[:], sr_f[:])  # cast to bf16

            if cfg.n_cores > 1:
                nc.gpsimd.collective_compute(
                    "AllGather", mybir.AluOpType.bypass, replica_groups=rg,
                    ins=[st_shard[:, :].opt()], outs=[st_full[:, :].opt()])
                st_src = st_full
            else:
                st_src = st_shard

            # combined [ent | s_t] gather table.  DMA row counts must stay
            # under the 16-bit ISA field limit -> copy in row-slices.
            def big_copy(dst, src_ap, rows):
                step = 50176
                for r0 in range(0, rows, step):
                    r1 = min(r0 + step, rows)
                    nc.sync.dma_start(dst(r0, r1), src_ap(r0, r1))

            big_copy(lambda a, b: comb[a:b, :],
                     lambda a, b: ent68_d[a:b, :], cfg.npt)
            big_copy(lambda a, b: comb[a:b, D:CR],
                     lambda a, b: st_src[a:b, :], cfg.npt)

            if cfg.debug:
                nc.sync.dma_start(dbg_st[:, :], st_src[:, :])
                nc.sync.dma_start(dbg_sh[:, :], sh_all[:])

            import concourse.bass as _b

            # ---- power iterations ----
            for k in range(1, cfg.pow_iter + 1):
                first = k == 1
                last = k == cfg.pow_iter
                src = comb if first else z_full[(k - 2) % 2]
                rowlen = CR if first else HDc
                for i in range(NT):
                    zg = zgp.tile([P, CH * HDc], f32, tag="zg")
                    for j in range(CH):
                        nc.gpsimd.indirect_dma_start(
                            out=zg[:, j * rowlen:(j + 1) * rowlen],
                            out_offset=None, in_=src[:, :],
                            in_offset=_b.IndirectOffsetOnAxis(
                                ap=tti_t[:, i * CH + j:i * CH + j + 1],
                                axis=0))
                    # one-hot S^T for all chunks of this tile: [P, CH*P]
                    s6 = sop.tile([P, CH * P], f32, tag="s")
                    nc.vector.tensor_tensor(
                        out=s6[:].rearrange("p (c n) -> p c n", c=CH),
                        in0=(hrel_t[:, i * CH:(i + 1) * CH]
                             .rearrange("p (c o) -> p c o", o=1)
                             .to_broadcast([P, CH, P])),
                        in1=(iota_f[:].rearrange("p (o n) -> p o n", o=1)
                             .to_broadcast([P, CH, P])),
                        op=OP.is_equal)
                    ps = ppA.tile([P, HDc], f32, tag="mm")
                    if first:
                        psd = ppD.tile([P, H], f32, tag="den")
                        for j in range(CH):
                            c = i * CH + j
                            # s_h: transpose S^T chunk, matmul vs sh_all
                            tp = ppB.tile([P, P], f32, tag="tp")
                            nc.tensor.transpose(
                                out=tp[:], in_=s6[:, j * P:(j + 1) * P],
                                identity=ident[:])
                            s2 = wk.tile([P, P], f32, tag="s2")
                            nc.vector.tensor_copy(s2[:], tp[:])
                            shr = ppC.tile([P, H], f32, tag="shr")
                            nc.tensor.matmul(
                                shr[:, :], lhsT=s2[:],
                                rhs=sh_all[:, i * H:(i + 1) * H],
                                start=True, stop=False)
                            for b in range(RB):
                                r1t = r1p.tile([P, P], bf16, tag="r1")
                                nc.sync.dma_start(
                                    r1t[:], r1h_d[b * P:(b + 1) * P,
                                                  c * P:(c + 1) * P])
                                nc.tensor.matmul(
                                    shr[:, :], lhsT=r1t[:],
                                    rhs=sr_b[:, b * H:(b + 1) * H],
                                    start=False, stop=(b == RB - 1))
                            # score = shr + s_t(from gather); leaky; exp
                            sc = sm.tile([P, H], f32, tag="sc")
                            nc.vector.tensor_tensor(
                                out=sc[:], in0=shr[:, :],
                                in1=zg[:, j * CR + D:(j + 1) * CR],
                                op=OP.add)
                            sc2 = sm.tile([P, H], f32, tag="sc2")
                            nc.vector.tensor_scalar_mul(sc2[:], sc[:],
                                                        cfg.leaky)
                            nc.vector.tensor_tensor(out=sc[:], in0=sc[:],
                                                    in1=sc2[:], op=OP.max)
                            nc.scalar.activation(
                                w_sb[:, c * H:(c + 1) * H], sc[:], AF.Exp)
                            nc.tensor.matmul(
                                psd[:, :], lhsT=s6[:, j * P:(j + 1) * P],
                                rhs=w_sb[:, c * H:(c + 1) * H],
                                start=(j == 0), stop=(j == CH - 1))
                    # messages for the whole tile, then U matmuls
                    msg = msgp.tile([P, CH * HDc], f32, tag="msg")
                    wap = (w_sb[:, i * CH * H:(i + 1) * CH * H]
                           .rearrange("p (c h o) -> p c h o", c=CH, h=H)
                           .to_broadcast([P, CH, H, D]))
                    if first:
                        zs = (zg[:, :CH * CR]
                              .rearrange("p (c r) -> p c r", c=CH)
                              [:, :, 0:D]
                              .rearrange("p c (o d) -> p c o d", o=1)
                              .to_broadcast([P, CH, H, D]))
                    else:
                        zs = zg[:].rearrange("p (c h d) -> p c h d",
                                             c=CH, h=H)
                    nc.vector.tensor_tensor(
                        out=msg[:].rearrange("p (c h d) -> p c h d",
                                             c=CH, h=H),
                        in0=zs, in1=wap, op=OP.mult)
                    for j in range(CH):
                        nc.tensor.matmul(
                            ps[:, :], lhsT=s6[:, j * P:(j + 1) * P],
                            rhs=msg[:, j * HDc:(j + 1) * HDc],
                            start=(j == 0), stop=(j == CH - 1))
                    if first:
                        d1 = sm.tile([P, H], f32, tag="d1")
                        nc.vector.tensor_scalar_add(d1[:], psd[:, :], cfg.eps)
                        d2 = sm.tile([P, H], f32, tag="d2")
                        nc.vector.reciprocal(d2[:], d1[:])
                        nc.vector.tensor_scalar_mul(
                            inv_sb[:, i * H:(i + 1) * H], d2[:],
                            1.0 - cfg.alpha)
                    # epilogue: zn = ps * inv + alpha * ent
                    zn = wk.tile([P, HDc], f32, tag="zn")
                    inv_b = (inv_sb[:, i * H:(i + 1) * H]
                             .rearrange("p (h o) -> p h o", o=1)
                             .to_broadcast([P, H, D]))
                    nc.vector.tensor_tensor(
                        out=zn[:].rearrange("p (h d) -> p h d", h=H),
                        in0=ps[:, :].rearrange("p (h d) -> p h d", h=H),
                        in1=inv_b, op=OP.mult)
                    ent = wk.tile([P, D], f32, tag="ent")
                    nc.sync.dma_start(ent[:], ent_shard_d[i * P:(i + 1) * P, :])
                    enta = wk.tile([P, D], f32, tag="enta")
                    nc.scalar.activation(enta[:], ent[:], AF.Copy,
                                         scale=cfg.alpha)
                    ent_b = (enta[:].rearrange("p (o d) -> p o d", o=1)
                             .to_broadcast([P, H, D]))
                    zn3 = zn[:].rearrange("p (h d) -> p h d", h=H)
                    nc.vector.tensor_tensor(out=zn3, in0=zn3, in1=ent_b,
                                            op=OP.add)
                    if cfg.debug and first and i == 0:
                        nc.sync.dma_start(dbg_zg[:, :], zg[:, :CH * CR])
                    if not last:
                        nc.sync.dma_start(z_shard[i * P:(i + 1) * P, :], zn[:])
                    else:
                        po = ppD.tile([P, D], f32, tag="out")
                        for b in range(HDc // P):
                            tpp = ppB.tile([P, P], f32, tag="tp")
                            nc.tensor.transpose(out=tpp[:],
                                                in_=zn[:, b * P:(b + 1) * P],
                                                identity=ident[:])
                            tps = wk.tile([P, P], f32, tag="tps")
                            nc.vector.tensor_copy(tps[:], tpp[:])
                            nc.tensor.matmul(po[:, :], lhsT=tps[:],
                                             rhs=wo_t[:, b * D:(b + 1) * D],
                                             start=(b == 0),
                                             stop=(b == HDc // P - 1))
                        ob = wk.tile([P, D], f32, tag="ob")
                        nc.vector.tensor_copy(ob[:], po[:, :])
                        nc.sync.dma_start(out_d[i * P:(i + 1) * P, :], ob[:])
                if not last and cfg.n_cores > 1:
                    nc.gpsimd.collective_compute(
                        "AllGather", mybir.AluOpType.bypass, replica_groups=rg,
                        ins=[z_shard[:, :].opt()],
                        outs=[z_full[(k - 1) % 2][:, :].opt()])
                elif not last:
                    nc.sync.dma_start(z_full[(k - 1) % 2][:, :], z_shard[:, :])
                if cfg.debug and first:
                    nc.sync.dma_start(dbg_w[:, :], w_sb[:])
                    nc.sync.dma_start(dbg_inv[:, :], inv_sb[:])
                    if not last:
                        nc.sync.dma_start(dbg_z1[:, :],
                                          z_full[0][:, :])
    nc.compile()
    return nc


def make_in_maps(cfg, cores, ent68, rel_pad, W_h, W_t, W_r, att_h, att_t,
                 att_r, W_o):
    def rep(att):
        return np.tile(np.asarray(att, np.float32).reshape(1, cfg.hd), (P, 1))

    common = dict(
        ent68=np.ascontiguousarray(ent68),
        rel_pad=np.ascontiguousarray(rel_pad),
        wh=np.ascontiguousarray(np.asarray(W_h, np.float32)),
        wt=np.ascontiguousarray(np.asarray(W_t, np.float32)),
        wr=np.ascontiguousarray(np.asarray(W_r, np.float32)),
        wo=np.ascontiguousarray(np.asarray(W_o, np.float32)),
        atth=rep(att_h), attt=rep(att_t), attr=rep(att_r),
    )
    in_maps = []
    for c in range(cfg.n_cores):
        m = dict(common)
        m["ent_shard"] = np.ascontiguousarray(cores[c]["ent_shard"])
        m["h_rel"] = np.ascontiguousarray(cores[c]["h_rel"])
        m["tti"] = np.ascontiguousarray(cores[c]["tti"])
        m["r1h"] = np.ascontiguousarray(cores[c]["r1h"])
        in_maps.append(m)
    return in_maps


_CACHE = {}


def kernel(entity_embed, relation_embed, W_h, W_t, W_r, att_h, att_t, att_r,
           W_o, edge_index, edge_type):
    from concourse.bass_utils import run_bass_kernel_spmd

    cfg = Cfg(NCORES, N_ENT, DIM, HEADS, N_REL, POW_ITER, 768)
    cfg, cores, ent68, rel_pad = host_prep(
        cfg, entity_embed, relation_embed, edge_index, edge_type)
    in_maps = make_in_maps(cfg, cores, ent68, rel_pad, W_h, W_t, W_r,
                           att_h, att_t, att_r, W_o)
    key = (cfg.n_cores, cfg.nps, cfg.l_cap)
    if key not in _CACHE:
        _CACHE[key] = build_program(cfg)
    nc = _CACHE[key]
    res = run_bass_kernel_spmd(nc, in_maps, core_ids=list(range(cfg.n_cores)))
    out = np.concatenate(
        [res.results[c]["out"][:cfg.npc] for c in range(cfg.n_cores)], axis=0)
    return out.astype(np.float32)



# revision 17
# speedup vs baseline: 1.5409x; 1.5409x over previous
"""DAGNConv (GNN message passing) Trainium2 kernel.

Strategy (8 NeuronCores, SPMD):
  - Host bin-packs nodes into 784 tiles of <=128 nodes (LPT on degree) so
    every tile holds ~640 edges -> l_cap=640 (5 chunks of 128 edge slots).
    Each core owns 98 tiles; segment softmax is core-local.
  - Scores (iter 1): per-node tables s_h/s_t (device matmul over entity
    tiles) and s_r (relation tiles).  Per-edge assembly:
      s_t rides the tail gather (combined [ent|s_t] bf16 row),
      s_h via host-shipped transposed head one-hot (bf16 matmul),
      s_r via host-shipped relation one-hot (bf16 matmul).
    Softmax denominator folded into the message matmul (4 extra columns);
    attention normalization folded into a per-node scale.
  - Power iterations: per 128-edge chunk, indirect-DMA row gather of
    Z[tails] (one row per partition), then segment-sum via one-hot matmul
    built on device (is_equal vs iota).  All Z state bf16.
  - Between iterations: the [12544, 256] Z shard AllGather is split into
    7 group collectives (14 tiles each) so they overlap tile compute.
  - Output: Z5 @ W_o folded into iteration 5 (PE transpose + matmul).
"""

import os
import sys

import numpy as np

for _p in ("/opt/trn_rl_repo",):
    if _p not in sys.path and os.path.isdir(_p):
        sys.path.insert(0, _p)

P = 128
N_ENT = 100000
N_EDGE = 500000
N_REL = 200
DIM = 64
HEADS = 4
HD = HEADS * DIM  # 256
CROW = DIM + HEADS  # combined [ent | s_t] row length (68)
POW_ITER = 5
ALPHA = 0.1
LEAKY = 0.01
EPS = 1e-16
NCORES = 8
NTILES = 98          # tiles per core
NBINS = NCORES * NTILES
NPS = NTILES * P     # padded nodes per core (12544)
NPT = NCORES * NPS   # padded total nodes (100352)
NGROUP = 7           # collective groups per iteration
GT = NTILES // NGROUP  # tiles per group (14)
GR = GT * P           # rows per group per core (1792)


class Cfg:
    def __init__(self, l_cap):
        assert l_cap % P == 0
        self.l_cap = l_cap
        self.ch = l_cap // P          # chunks per tile
        self.nchunk = NTILES * self.ch  # chunks per core


def _balance_nodes(heads):
    """LPT bin-packing of nodes into NBINS tiles (<=128 nodes each),
    minimizing max edges per tile.  Returns newpos[orig_node] (global
    padded row) and per-bin node lists."""
    import heapq

    deg = np.bincount(heads, minlength=N_ENT)
    order = np.argsort(-deg, kind="stable")
    heap = [(0, b) for b in range(NBINS)]
    heapq.heapify(heap)
    bin_nodes = [[] for _ in range(NBINS)]
    bin_load = np.zeros(NBINS, np.int64)
    for n in order:
        d = int(deg[n])
        while True:
            load, b = heapq.heappop(heap)
            if len(bin_nodes[b]) < P:
                bin_nodes[b].append(n)
                bin_load[b] = load + d
                heapq.heappush(heap, (load + d, b))
                break
            # full bins simply stay out of the heap
    newpos = np.empty(N_ENT, np.int64)
    for b in range(NBINS):
        for i, n in enumerate(bin_nodes[b]):
            newpos[n] = b * P + i
    return newpos, int(bin_load.max())


def host_prep(cfg, entity_embed, relation_embed, edge_index, edge_type):
    import ml_dtypes

    bf16 = ml_dtypes.bfloat16
    h = np.asarray(edge_index[0], dtype=np.int64)
    t = np.asarray(edge_index[1], dtype=np.int64)
    r = np.asarray(edge_type, dtype=np.int64)

    newpos, max_load = _balance_nodes(h)
    l_cap = -(-max_load // P) * P
    if l_cap != cfg.l_cap:
        cfg = Cfg(l_cap)
    CH = cfg.ch

    # per-edge new head row, sorted so each tile's edges are contiguous
    hn = newpos[h]
    perm = np.argsort(hn, kind="stable")
    hs, ts, rs = hn[perm], t[perm], r[perm]
    tn = newpos[ts]  # new padded tail row (plain layout)

    # group-major z_full row for each tail
    cc = tn // NPS
    ll = tn % NPS
    tz = (ll // GR) * (NCORES * GR) + cc * GR + (ll % GR)

    ent = np.asarray(entity_embed, dtype=np.float32)
    ent_new = np.zeros((NPT, DIM), np.float32)
    ent_new[newpos] = ent
    ent68b = np.zeros((NPT, CROW), bf16)
    ent68b[:, :DIM] = ent_new.astype(bf16)
    relpad = np.zeros((2 * P, DIM), np.float32)
    relpad[:N_REL] = np.asarray(relation_embed, np.float32)
    relpad_b = relpad.astype(bf16)

    cores = []
    tile_of_edge = hs // P          # global tile id per edge
    # slot within tile
    tile_start = np.searchsorted(tile_of_edge, np.arange(NBINS))
    for c in range(NCORES):
        hrel = np.full((P, cfg.nchunk), -1.0, np.float32)
        tti_e = np.zeros((P, cfg.nchunk), np.int32)
        tti_z = np.zeros((P, cfg.nchunk), np.int32)
        h1h = np.zeros((P, cfg.nchunk * P), bf16)
        r1h = np.zeros((2 * P, cfg.nchunk * P), bf16)
        for i in range(NTILES):
            b = c * NTILES + i
            lo = int(tile_start[b])
            hi = int(tile_start[b + 1]) if b + 1 < NBINS else len(hs)
            cnt = hi - lo
            if cnt == 0:
                continue
            j = np.arange(cnt)
            cols = i * CH + (j // P)
            parts = j % P
            hloc = (hs[lo:hi] - b * P).astype(np.int64)
            hrel[parts, cols] = hloc.astype(np.float32)
            tti_e[parts, cols] = tn[lo:hi].astype(np.int32)
            tti_z[parts, cols] = tz[lo:hi].astype(np.int32)
            h1h[hloc, cols * P + parts] = 1.0
            r1h[rs[lo:hi], cols * P + parts] = 1.0
        entloc = ent_new[c * NPS:(c + 1) * NPS].astype(bf16)
        cores.append(dict(hrel=hrel, tti_e=tti_e, tti_z=tti_z,
                          h1h=h1h, r1h=r1h, entloc=entloc))
    return cfg, cores, ent68b, relpad_b, newpos


def build_program(cfg):
    import concourse.bass as bass
    import concourse.bacc as bacc
    import concourse.mybir as mybir
    from concourse.masks import make_identity
    from concourse.tile import TileContext

    f32 = mybir.dt.float32
    i32 = mybir.dt.int32
    bf16 = mybir.dt.bfloat16
    AF = mybir.ActivationFunctionType
    OP = mybir.AluOpType
    AX = mybir.AxisListType
    H, D = HEADS, DIM
    CH = cfg.ch
    NCK = cfg.nchunk
    MW = HD + H  # message matmul width in iter 1 (260)

    nc = bacc.Bacc("TRN2", target_bir_lowering=False, debug=False,
                   num_devices=NCORES)

    # ---- I/O ----
    ent68_d = nc.dram_tensor("ent68b", [NPT, CROW], bf16, kind="ExternalInput")
    entloc_d = nc.dram_tensor("entloc", [NPS, D], bf16, kind="ExternalInput")
    relpad_d = nc.dram_tensor("relpad", [2 * P, D], bf16, kind="ExternalInput")
    wht_d = nc.dram_tensor("wht", [D, 2 * HD], bf16, kind="ExternalInput")
    wr_d = nc.dram_tensor("wr", [D, HD], bf16, kind="ExternalInput")
    wo_d = nc.dram_tensor("wo", [P, 2 * D], bf16, kind="ExternalInput")
    attht_d = nc.dram_tensor("attht", [P, 2 * HD], bf16, kind="ExternalInput")
    attr_d = nc.dram_tensor("attr", [P, HD], bf16, kind="ExternalInput")
    hrel_d = nc.dram_tensor("hrel", [P, NCK], f32, kind="ExternalInput")
    ttie_d = nc.dram_tensor("tti_e", [P, NCK], i32, kind="ExternalInput")
    ttiz_d = nc.dram_tensor("tti_z", [P, NCK], i32, kind="ExternalInput")
    h1h_d = nc.dram_tensor("h1h", [P, NCK * P], bf16, kind="ExternalInput")
    r1h_d = nc.dram_tensor("r1h", [2 * P, NCK * P], bf16, kind="ExternalInput")
    out_d = nc.dram_tensor("out", [NPS, D], f32, kind="ExternalOutput")

    # ---- internal DRAM ----
    comb = nc.dram_tensor("comb", [NPT, CROW], bf16)
    z_shard = nc.dram_tensor("z_shard", [NPS, HD], bf16)
    z_full = [nc.dram_tensor(f"z_full{i}", [NPT, HD], bf16,
                             addr_space="Shared") for i in range(2)]
    st_shard = nc.dram_tensor("st_shard", [NPS, H], bf16)
    st_full = nc.dram_tensor("st_full", [NPT, H], bf16, addr_space="Shared")
    rg = [list(range(NCORES))]

    with TileContext(nc) as tc:
        with (
            tc.tile_pool(name="const", bufs=1) as cp,
            tc.tile_pool(name="work", bufs=3) as wk,
            tc.tile_pool(name="small", bufs=4) as sm,
            tc.tile_pool(name="zg", bufs=4) as zgp,
            tc.tile_pool(name="msg", bufs=3) as msgp,
            tc.tile_pool(name="sone", bufs=3) as sop,
            tc.tile_pool(name="oneh", bufs=4) as ohp,
            tc.tile_pool(name="ppA", bufs=2, space="PSUM") as ppA,
            tc.tile_pool(name="ppB", bufs=2, space="PSUM") as ppB,
            tc.tile_pool(name="ppC", bufs=2, space="PSUM") as ppC,
            tc.tile_pool(name="ppD", bufs=2, space="PSUM") as ppD,
        ):
            # ---- constants ----
            ident = cp.tile([P, P], f32, tag="ident")
            make_identity(nc, ident[:])
            identb = cp.tile([P, P], bf16, tag="identb")
            make_identity(nc, identb[:])
            iota_i = cp.tile([P, P], i32, tag="iota_i")
            nc.gpsimd.iota(iota_i[:], pattern=[[1, P]], base=0,
                           channel_multiplier=0)
            iota_f = cp.tile([P, P], f32, tag="iota_f")
            nc.vector.tensor_copy(iota_f[:], iota_i[:])

            def load_const(dram, shape, tag, dt=bf16):
                t = cp.tile(shape, dt, tag=tag)
                nc.sync.dma_start(t[:], dram[:, :])
                return t

            wht_t = load_const(wht_d, [D, 2 * HD], "wht")
            wr_t = load_const(wr_d, [D, HD], "wr")
            wo_t = load_const(wo_d, [P, 2 * D], "wo")
            attht_t = load_const(attht_d, [P, 2 * HD], "attht")
            attr_t = load_const(attr_d, [P, HD], "attr")
            hrel_t = load_const(hrel_d, [P, NCK], "hrel", f32)
            ttie_t = load_const(ttie_d, [P, NCK], "ttie", i32)
            ttiz_t = load_const(ttiz_d, [P, NCK], "ttiz", i32)

            w_sb = cp.tile([P, NCK * H], bf16, tag="w")
            inv_sb = cp.tile([P, NTILES * H], f32, tag="inv")
            sh_all = cp.tile([P, NTILES * H], bf16, tag="sh_all")
            sr_b = cp.tile([P, 2 * H], bf16, tag="sr_b")
            aent = cp.tile([P, NTILES * D], bf16, tag="aent")

            # ---- combined [ent | s_t] table: bulk ent copy ----
            step = 50176
            for r0 in range(0, NPT, step):
                r1 = min(r0 + step, NPT)
                nc.sync.dma_start(comb[r0:r1, :], ent68_d[r0:r1, :])

            # ---- phase 1: node/relation score tables ----
            def table_pass(src_d, n_tiles, W_t, att_t, width, sink):
                for i in range(n_tiles):
                    ent = wk.tile([P, D], bf16, tag="ent")
                    nc.sync.dma_start(ent[:], src_d[i * P:(i + 1) * P, :])
                    tp = ppB.tile([P, P], bf16, tag="tp")
                    nc.tensor.transpose(out=tp[:D, :], in_=ent[:, :],
                                        identity=identb[:])
                    entT = wk.tile([P, P], bf16, tag="entT")
                    nc.scalar.activation(entT[:D, :], tp[:D, :], AF.Copy)
                    pj = ppA.tile([P, 2 * HD], f32, tag="mm")
                    nc.tensor.matmul(pj[:, :width], lhsT=entT[:D, :],
                                     rhs=W_t[:, :], start=True, stop=True)
                    th_ = wk.tile([P, 2 * HD], bf16, tag="tanh")
                    nc.scalar.activation(th_[:, :width], pj[:, :width],
                                         AF.Tanh)
                    pr = wk.tile([P, 2 * HD], f32, tag="prod")
                    nc.vector.tensor_tensor(out=pr[:, :width],
                                            in0=th_[:, :width],
                                            in1=att_t[:], op=OP.mult)
                    s_ = sm.tile([P, 2 * H], f32, tag="s8")
                    nc.vector.tensor_reduce(
                        out=s_[:, :width // D],
                        in_=pr[:, :width].rearrange("p (h d) -> p h d", d=D),
                        axis=AX.X, op=OP.add)
                    sink(i, ent, s_)
            def ent_sink(i, ent, s8):
                nc.scalar.activation(aent[:, i * D:(i + 1) * D], ent[:],
                                     AF.Copy, scale=ALPHA)
                nc.vector.tensor_copy(sh_all[:, i * H:(i + 1) * H],
                                      s8[:, 0:H])
                st4 = sm.tile([P, H], bf16, tag="st4")
                nc.vector.tensor_copy(st4[:], s8[:, H:2 * H])
                nc.sync.dma_start(st_shard[i * P:(i + 1) * P, :], st4[:])

            def rel_sink(i, ent, s4):
                nc.vector.tensor_copy(sr_b[:, i * H:(i + 1) * H], s4[:, 0:H])

            table_pass(entloc_d, NTILES, wht_t, attht_t, 2 * HD, ent_sink)
            table_pass(relpad_d, 2, wr_t, attr_t, HD, rel_sink)

            nc.gpsimd.collective_compute(
                "AllGather", mybir.AluOpType.bypass, replica_groups=rg,
                ins=[st_shard[:, :].opt()], outs=[st_full[:, :].opt()])
            for r0 in range(0, NPT, step):
                r1 = min(r0 + step, NPT)
                nc.sync.dma_start(comb[r0:r1, D:CROW], st_full[r0:r1, :])

            import concourse.bass as _b

            # ---- power iterations ----
            for k in range(1, POW_ITER + 1):
                first = k == 1
                last = k == POW_ITER
                src = comb if first else z_full[(k - 2) % 2]
                rowlen = CROW if first else HD
                tti = ttie_t if first else ttiz_t
                for i in range(NTILES):
                    zg = zgp.tile([P, CH * rowlen], bf16, tag="zg")
                    for j in range(CH):
                        nc.gpsimd.indirect_dma_start(
                            out=zg[:, j * rowlen:(j + 1) * rowlen],
                            out_offset=None, in_=src[:, :],
                            in_offset=_b.IndirectOffsetOnAxis(
                                ap=tti[:, i * CH + j:i * CH + j + 1],
                                axis=0))
                    # one-hot S^T for all chunks of this tile: [P, CH*P]
                    s6 = sop.tile([P, CH * P], bf16, tag="s")
                    nc.vector.tensor_tensor(
                        out=s6[:].rearrange("p (c n) -> p c n", c=CH),
                        in0=(hrel_t[:, i * CH:(i + 1) * CH]
                             .rearrange("p (c o) -> p c o", o=1)
                             .to_broadcast([P, CH, P])),
                        in1=(iota_f[:].rearrange("p (o n) -> p o n", o=1)
                             .to_broadcast([P, CH, P])),
                        op=OP.is_equal)
                    mw = MW if first else HD
                    if first:
                        # per-edge scores: s_h + s_r via one-hot matmuls
                        shr = ppC.tile([P, D], f32, tag="shr")
                        for j in range(CH):
                            c = i * CH + j
                            oh = ohp.tile([P, P], bf16, tag="oh")
                            nc.sync.dma_start(
                                oh[:], h1h_d[:, c * P:(c + 1) * P])
                            nc.tensor.matmul(
                                shr[:, j * H:(j + 1) * H], lhsT=oh[:],
                                rhs=sh_all[:, i * H:(i + 1) * H],
                                start=True, stop=False)
                            for b in range(2):
                                r1t = ohp.tile([P, P], bf16, tag="r1")
                                nc.sync.dma_start(
                                    r1t[:], r1h_d[b * P:(b + 1) * P,
                                                  c * P:(c + 1) * P])
                                nc.tensor.matmul(
                                    shr[:, j * H:(j + 1) * H], lhsT=r1t[:],
                                    rhs=sr_b[:, b * H:(b + 1) * H],
                                    start=False, stop=(b == 1))
                        # scores for whole tile: add s_t, leaky, exp
                        stf = sm.tile([P, CH * H], f32, tag="stf")
                        nc.vector.tensor_copy(
                            stf[:].rearrange("p (c h) -> p c h", c=CH),
                            zg[:].rearrange("p (c r) -> p c r", c=CH)
                            [:, :, D:CROW])
                        sc = sm.tile([P, CH * H], f32, tag="sc")
                        nc.vector.tensor_tensor(out=sc[:],
                                                in0=shr[:, 0:CH * H],
                                                in1=stf[:], op=OP.add)
                        sc2 = sm.tile([P, CH * H], f32, tag="sc2")
                        nc.vector.tensor_scalar_mul(sc2[:], sc[:], LEAKY)
                        nc.vector.tensor_tensor(out=sc[:], in0=sc[:],
                                                in1=sc2[:], op=OP.max)
                        nc.scalar.activation(
                            w_sb[:, i * CH * H:(i + 1) * CH * H], sc[:],
                            AF.Exp)
                    # messages for the whole tile
                    msg = msgp.tile([P, CH * MW], bf16, tag="msg")
                    wap = (w_sb[:, i * CH * H:(i + 1) * CH * H]
                           .rearrange("p (c h o) -> p c h o", c=CH, h=H)
                           .to_broadcast([P, CH, H, D]))
                    mview = (msg[:, 0:CH * mw]
                             .rearrange("p (c x) -> p c x", c=CH)
                             [:, :, 0:HD]
                             .rearrange("p c (h d) -> p c h d", h=H))
                    if first:
                        zs = (zg[:].rearrange("p (c r) -> p c r", c=CH)
                              [:, :, 0:D]
                              .rearrange("p c (o d) -> p c o d", o=1)
                              .to_broadcast([P, CH, H, D]))
                    else:
                        zs = zg[:].rearrange("p (c h d) -> p c h d",
                                             c=CH, h=H)
                    nc.vector.tensor_tensor(out=mview, in0=zs, in1=wap,
                                            op=OP.mult)
                    if first:
                        # denominator columns: msg[:, c*MW+HD : c*MW+MW] = w
                        nc.vector.tensor_copy(
                            (msg[:, 0:CH * MW]
                             .rearrange("p (c x) -> p c x", c=CH)
                             [:, :, HD:MW]),
                            (w_sb[:, i * CH * H:(i + 1) * CH * H]
                             .rearrange("p (c h) -> p c h", c=CH)))
                    ps = ppD.tile([P, MW], f32, tag="mm")
                    for j in range(CH):
                        nc.tensor.matmul(
                            ps[:, :mw], lhsT=s6[:, j * P:(j + 1) * P],
                            rhs=msg[:, j * mw:(j + 1) * mw],
                            start=(j == 0), stop=(j == CH - 1))
                    if first:
                        d1 = sm.tile([P, H], f32, tag="d1")
                        nc.vector.tensor_scalar_add(d1[:], ps[:, HD:MW], EPS)
                        d2 = sm.tile([P, H], f32, tag="d2")
                        nc.vector.reciprocal(d2[:], d1[:])
                        nc.vector.tensor_scalar_mul(
                            inv_sb[:, i * H:(i + 1) * H], d2[:], 1.0 - ALPHA)
                    # epilogue: zn = ps * inv + alpha * ent
                    zn = wk.tile([P, HD], bf16, tag="zn")
                    inv_b = (inv_sb[:, i * H:(i + 1) * H]
                             .rearrange("p (h o) -> p h o", o=1)
                             .to_broadcast([P, H, D]))
                    nc.vector.tensor_tensor(
                        out=zn[:].rearrange("p (h d) -> p h d", h=H),
                        in0=ps[:, 0:HD].rearrange("p (h d) -> p h d", h=H),
                        in1=inv_b, op=OP.mult)
                    ent_b = (aent[:, i * D:(i + 1) * D]
                             .rearrange("p (o d) -> p o d", o=1)
                             .to_broadcast([P, H, D]))
                    zn3 = zn[:].rearrange("p (h d) -> p h d", h=H)
                    nc.vector.tensor_tensor(out=zn3, in0=zn3, in1=ent_b,
                                            op=OP.add)
                    if not last:
                        nc.sync.dma_start(z_shard[i * P:(i + 1) * P, :],
                                          zn[:])
                        if i % GT == GT - 1:
                            g = i // GT
                            nc.gpsimd.collective_compute(
                                "AllGather", mybir.AluOpType.bypass,
                                replica_groups=rg,
                                ins=[z_shard[g * GR:(g + 1) * GR, :].opt()],
                                outs=[z_full[(k - 1) % 2]
                                      [g * NCORES * GR:(g + 1) * NCORES * GR,
                                       :].opt()])
                    else:
                        po = ppC.tile([P, D], f32, tag="shr")
                        for b in range(HD // P):
                            tpp = ppB.tile([P, P], bf16, tag="tp")
                            nc.tensor.transpose(out=tpp[:],
                                                in_=zn[:, b * P:(b + 1) * P],
                                                identity=identb[:])
                            tps = wk.tile([P, P], bf16, tag="tps")
                            nc.scalar.activation(tps[:], tpp[:], AF.Copy)
                            nc.tensor.matmul(po[:, :], lhsT=tps[:],
                                             rhs=wo_t[:, b * D:(b + 1) * D],
                                             start=(b == 0),
                                             stop=(b == HD // P - 1))
                        ob = wk.tile([P, D], f32, tag="ob")
                        nc.vector.tensor_copy(ob[:], po[:, :])
                        nc.sync.dma_start(out_d[i * P:(i + 1) * P, :], ob[:])
    nc.compile()
    return nc


def make_in_maps(cfg, cores, ent68b, relpad_b, W_h, W_t, W_r, att_h, att_t,
                 att_r, W_o):
    import ml_dtypes

    bf16 = ml_dtypes.bfloat16

    def rep(att, n):
        a = np.concatenate([np.asarray(x, np.float32).reshape(1, HD)
                            for x in att], axis=1)
        return np.tile(a, (P, 1)).astype(bf16)

    wht = np.concatenate([np.asarray(W_h, np.float32),
                          np.asarray(W_t, np.float32)], axis=1).astype(bf16)
    wo = np.asarray(W_o, np.float32)  # [256, 64]
    wo_b = np.concatenate([wo[:P, :], wo[P:, :]], axis=1).astype(bf16)
    common = dict(
        ent68b=np.ascontiguousarray(ent68b),
        relpad=np.ascontiguousarray(relpad_b),
        wht=np.ascontiguousarray(wht),
        wr=np.ascontiguousarray(np.asarray(W_r, np.float32).astype(bf16)),
        wo=np.ascontiguousarray(wo_b),
        attht=np.ascontiguousarray(rep([att_h, att_t], 2)),
        attr=np.ascontiguousarray(rep([att_r], 1)),
    )
    in_maps = []
    for c in range(NCORES):
        m = dict(common)
        m["entloc"] = np.ascontiguousarray(cores[c]["entloc"])
        m["hrel"] = np.ascontiguousarray(cores[c]["hrel"])
        m["tti_e"] = np.ascontiguousarray(cores[c]["tti_e"])
        m["tti_z"] = np.ascontiguousarray(cores[c]["tti_z"])
        m["h1h"] = np.ascontiguousarray(cores[c]["h1h"])
        m["r1h"] = np.ascontiguousarray(cores[c]["r1h"])
        in_maps.append(m)
    return in_maps


_CACHE = {}


def kernel(entity_embed, relation_embed, W_h, W_t, W_r, att_h, att_t, att_r,
           W_o, edge_index, edge_type):
    from concourse.bass_utils import run_bass_kernel_spmd

    cfg = Cfg(640)
    cfg, cores, ent68b, relpad_b, newpos = host_prep(
        cfg, entity_embed, relation_embed, edge_index, edge_type)
    in_maps = make_in_maps(cfg, cores, ent68b, relpad_b, W_h, W_t, W_r,
                           att_h, att_t, att_r, W_o)
    key = cfg.l_cap
    if key not in _CACHE:
        _CACHE[key] = build_program(cfg)
    nc = _CACHE[key]
    res = run_bass_kernel_spmd(nc, in_maps, core_ids=list(range(NCORES)))
    full = np.concatenate(
        [res.results[c]["out"] for c in range(NCORES)], axis=0)
    return full[newpos].astype(np.float32)


# revision 24
# speedup vs baseline: 1.8773x; 1.2183x over previous
"""DAGNConv (GNN message passing) Trainium2 kernel.

Strategy (8 NeuronCores, SPMD):
  - Host bin-packs nodes into 784 tiles of <=128 nodes (LPT on degree) so
    every tile holds ~640 edges -> l_cap=640 (5 chunks of 128 edge slots).
    Each core owns 98 tiles; segment softmax is core-local.
  - Scores (iter 1): per-node tables s_h/s_t (device matmul over entity
    tiles) and s_r (relation tiles).  Per-edge assembly:
      s_t rides the tail gather (combined [ent|s_t] bf16 row),
      s_h via host-shipped transposed head one-hot (bf16 matmul),
      s_r via host-shipped relation one-hot (bf16 matmul).
    Softmax denominator folded into the message matmul (4 extra columns);
    attention normalization folded into a per-node scale.
  - Power iterations: per 128-edge chunk, indirect-DMA row gather of
    Z[tails] (one row per partition), then segment-sum via one-hot matmul
    built on device (is_equal vs iota).  All Z state bf16.
  - Between iterations: the [12544, 256] Z shard AllGather is split into
    7 group collectives (14 tiles each) so they overlap tile compute.
  - Output: Z5 @ W_o folded into iteration 5 (PE transpose + matmul).
"""

import os
import sys

import numpy as np

for _p in ("/opt/trn_rl_repo",):
    if _p not in sys.path and os.path.isdir(_p):
        sys.path.insert(0, _p)

P = 128
N_ENT = 100000
N_EDGE = 500000
N_REL = 200
DIM = 64
HEADS = 4
HD = HEADS * DIM  # 256
CROW = DIM + HEADS  # combined [ent | s_t] row length (68)
POW_ITER = 5
ALPHA = 0.1
LEAKY = 0.01
EPS = 1e-16
NCORES = 8
NTILES = 98          # tiles per core
NBINS = NCORES * NTILES
NPS = NTILES * P     # padded nodes per core (12544)
NPT = NCORES * NPS   # padded total nodes (100352)
NGROUP = 7           # collective groups per iteration
GT = NTILES // NGROUP  # tiles per group (14)
GR = GT * P           # rows per group per core (1792)


class Cfg:
    def __init__(self, l_cap):
        assert l_cap % P == 0
        self.l_cap = l_cap
        self.ch = l_cap // P          # chunks per tile
        self.nchunk = NTILES * self.ch  # chunks per core


def _balance_nodes(heads):
    """LPT bin-packing of nodes into NBINS tiles (<=128 nodes each),
    minimizing max edges per tile.  Returns newpos[orig_node] (global
    padded row) and per-bin node lists."""
    import heapq

    deg = np.bincount(heads, minlength=N_ENT)
    order = np.argsort(-deg, kind="stable")
    heap = [(0, b) for b in range(NBINS)]
    heapq.heapify(heap)
    bin_nodes = [[] for _ in range(NBINS)]
    bin_load = np.zeros(NBINS, np.int64)
    for n in order:
        d = int(deg[n])
        while True:
            load, b = heapq.heappop(heap)
            if len(bin_nodes[b]) < P:
                bin_nodes[b].append(n)
                bin_load[b] = load + d
                heapq.heappush(heap, (load + d, b))
                break
            # full bins simply stay out of the heap
    newpos = np.empty(N_ENT, np.int64)
    for b in range(NBINS):
        for i, n in enumerate(bin_nodes[b]):
            newpos[n] = b * P + i
    return newpos, int(bin_load.max())


def host_prep(cfg, entity_embed, relation_embed, edge_index, edge_type):
    import ml_dtypes

    bf16 = ml_dtypes.bfloat16
    h = np.asarray(edge_index[0], dtype=np.int64)
    t = np.asarray(edge_index[1], dtype=np.int64)
    r = np.asarray(edge_type, dtype=np.int64)

    newpos, max_load = _balance_nodes(h)
    l_cap = -(-max_load // P) * P
    if l_cap != cfg.l_cap:
        cfg = Cfg(l_cap)
    CH = cfg.ch

    # per-edge new head row, sorted so each tile's edges are contiguous
    hn = newpos[h]
    perm = np.argsort(hn, kind="stable")
    hs, ts, rs = hn[perm], t[perm], r[perm]
    tn = newpos[ts]  # new padded tail row (plain layout)

    # group-major z_full row for each tail
    cc = tn // NPS
    ll = tn % NPS
    tz = (ll // GR) * (NCORES * GR) + cc * GR + (ll % GR)

    ent = np.asarray(entity_embed, dtype=np.float32)
    ent_new = np.zeros((NPT, DIM), np.float32)
    ent_new[newpos] = ent
    relpad = np.zeros((2 * P, DIM), np.float32)
    relpad[:N_REL] = np.asarray(relation_embed, np.float32)
    relpad_b = relpad.astype(bf16)

    cores = []
    tile_of_edge = hs // P          # global tile id per edge
    # slot within tile
    tile_start = np.searchsorted(tile_of_edge, np.arange(NBINS))
    for c in range(NCORES):
        hrel = np.full((P, cfg.nchunk), -1.0, np.float32)
        tti = np.zeros((P, cfg.nchunk), np.int32)
        h1h = np.zeros((P, cfg.nchunk * P), bf16)
        r1h = np.zeros((2 * P, cfg.nchunk * P), bf16)
        for i in range(NTILES):
            b = c * NTILES + i
            lo = int(tile_start[b])
            hi = int(tile_start[b + 1]) if b + 1 < NBINS else len(hs)
            cnt = hi - lo
            if cnt == 0:
                continue
            j = np.arange(cnt)
            cols = i * CH + (j // P)
            parts = j % P
            hloc = (hs[lo:hi] - b * P).astype(np.int64)
            hrel[parts, cols] = hloc.astype(np.float32)
            tti[parts, cols] = tz[lo:hi].astype(np.int32)
            h1h[hloc, cols * P + parts] = 1.0
            r1h[rs[lo:hi], cols * P + parts] = 1.0
        entloc = ent_new[c * NPS:(c + 1) * NPS].astype(bf16)
        cores.append(dict(hrel=hrel, tti=tti,
                          h1h=h1h, r1h=r1h, entloc=entloc))
    return cfg, cores, relpad_b, newpos


def build_program(cfg):
    import concourse.bass as bass
    import concourse.bacc as bacc
    import concourse.mybir as mybir
    from concourse.masks import make_identity
    from concourse.tile import TileContext

    f32 = mybir.dt.float32
    i32 = mybir.dt.int32
    bf16 = mybir.dt.bfloat16
    AF = mybir.ActivationFunctionType
    OP = mybir.AluOpType
    AX = mybir.AxisListType
    H, D = HEADS, DIM
    CH = cfg.ch
    NCK = cfg.nchunk
    MW = HD + H  # message matmul width in iter 1 (260)

    nc = bacc.Bacc("TRN2", target_bir_lowering=False, debug=False,
                   num_devices=NCORES)

    # ---- I/O ----
    entloc_d = nc.dram_tensor("entloc", [NPS, D], bf16, kind="ExternalInput")
    relpad_d = nc.dram_tensor("relpad", [2 * P, D], bf16, kind="ExternalInput")
    wht_d = nc.dram_tensor("wht", [D, 2 * HD], bf16, kind="ExternalInput")
    wr_d = nc.dram_tensor("wr", [D, HD], bf16, kind="ExternalInput")
    wo_d = nc.dram_tensor("wo", [P, 2 * D], bf16, kind="ExternalInput")
    attht_d = nc.dram_tensor("attht", [P, 2 * HD], bf16, kind="ExternalInput")
    attr_d = nc.dram_tensor("attr", [P, HD], bf16, kind="ExternalInput")
    hrel_d = nc.dram_tensor("hrel", [P, NCK], f32, kind="ExternalInput")
    tti_d = nc.dram_tensor("tti", [P, NCK], i32, kind="ExternalInput")
    h1h_d = nc.dram_tensor("h1h", [P, NCK * P], bf16, kind="ExternalInput")
    r1h_d = nc.dram_tensor("r1h", [2 * P, NCK * P], bf16, kind="ExternalInput")
    out_d = nc.dram_tensor("out", [NPS, D], f32, kind="ExternalOutput")

    # ---- internal DRAM ----
    # comb rows are group-major (same layout as z_full) so the per-group
    # [ent|s_t] AllGather writes land contiguously.
    comb = nc.dram_tensor("comb", [NPT, CROW], bf16, addr_space="Shared")
    st68 = nc.dram_tensor("st68", [NPS, CROW], bf16)
    z_shard = nc.dram_tensor("z_shard", [NPS, HD], bf16)
    z_full = [nc.dram_tensor(f"z_full{i}", [NPT, HD], bf16,
                             addr_space="Shared") for i in range(2)]
    rg = [list(range(NCORES))]

    with TileContext(nc) as tc:
        with (
            tc.tile_pool(name="const", bufs=1) as cp,
            tc.tile_pool(name="work", bufs=3) as wk,
            tc.tile_pool(name="small", bufs=4) as sm,
            tc.tile_pool(name="zg", bufs=4) as zgp,
            tc.tile_pool(name="msg", bufs=3) as msgp,
            tc.tile_pool(name="sone", bufs=3) as sop,
            tc.tile_pool(name="oneh", bufs=4) as ohp,
            tc.tile_pool(name="ppA", bufs=2, space="PSUM") as ppA,
            tc.tile_pool(name="ppB", bufs=2, space="PSUM") as ppB,
            tc.tile_pool(name="ppC", bufs=2, space="PSUM") as ppC,
            tc.tile_pool(name="ppD", bufs=2, space="PSUM") as ppD,
        ):
            # ---- constants ----
            ident = cp.tile([P, P], f32, tag="ident")
            make_identity(nc, ident[:])
            identb = cp.tile([P, P], bf16, tag="identb")
            make_identity(nc, identb[:])
            iota_i = cp.tile([P, P], i32, tag="iota_i")
            nc.gpsimd.iota(iota_i[:], pattern=[[1, P]], base=0,
                           channel_multiplier=0)
            iota_f = cp.tile([P, P], f32, tag="iota_f")
            nc.vector.tensor_copy(iota_f[:], iota_i[:])

            def load_const(dram, shape, tag, dt=bf16):
                t = cp.tile(shape, dt, tag=tag)
                nc.sync.dma_start(t[:], dram[:, :])
                return t

            wht_t = load_const(wht_d, [D, 2 * HD], "wht")
            wr_t = load_const(wr_d, [D, HD], "wr")
            wo_t = load_const(wo_d, [P, 2 * D], "wo")
            attht_t = load_const(attht_d, [P, 2 * HD], "attht")
            attr_t = load_const(attr_d, [P, HD], "attr")
            hrel_t = load_const(hrel_d, [P, NCK], "hrel", f32)
            tti_t = load_const(tti_d, [P, NCK], "tti", i32)

            w_sb = cp.tile([P, NCK * H], bf16, tag="w")
            inv_sb = cp.tile([P, NTILES * H], f32, tag="inv")
            sh_all = cp.tile([P, NTILES * H], bf16, tag="sh_all")
            sr_b = cp.tile([P, 2 * H], bf16, tag="sr_b")
            aent = cp.tile([P, NTILES * D], bf16, tag="aent")

            # ---- phase 1: node/relation score tables ----
            def table_pass(src_d, n_tiles, W_t, att_t, width, sink):
                for i in range(n_tiles):
                    ent = wk.tile([P, D], bf16, tag="ent")
                    nc.sync.dma_start(ent[:], src_d[i * P:(i + 1) * P, :])
                    tp = ppB.tile([P, P], bf16, tag="tp")
                    nc.tensor.transpose(out=tp[:D, :], in_=ent[:, :],
                                        identity=identb[:])
                    entT = wk.tile([P, P], bf16, tag="entT")
                    nc.scalar.activation(entT[:D, :], tp[:D, :], AF.Copy)
                    pj = ppA.tile([P, 2 * HD], f32, tag="mm")
                    nc.tensor.matmul(pj[:, :width], lhsT=entT[:D, :],
                                     rhs=W_t[:, :], start=True, stop=True)
                    th_ = wk.tile([P, 2 * HD], bf16, tag="tanh")
                    nc.scalar.activation(th_[:, :width], pj[:, :width],
                                         AF.Tanh)
                    pr = wk.tile([P, 2 * HD], f32, tag="prod")
                    nc.vector.tensor_tensor(out=pr[:, :width],
                                            in0=th_[:, :width],
                                            in1=att_t[:], op=OP.mult)
                    s_ = sm.tile([P, 2 * H], f32, tag="s8")
                    nc.vector.tensor_reduce(
                        out=s_[:, :width // D],
                        in_=pr[:, :width].rearrange("p (h d) -> p h d", d=D),
                        axis=AX.X, op=OP.add)
                    sink(i, ent, s_)

            def ent_sink(i, ent, s8):
                nc.scalar.activation(aent[:, i * D:(i + 1) * D], ent[:],
                                     AF.Copy, scale=ALPHA)
                nc.vector.tensor_copy(sh_all[:, i * H:(i + 1) * H],
                                      s8[:, 0:H])
                # assemble [ent | s_t] row block, ship to st68 shard
                cb = wk.tile([P, CROW], bf16, tag="cb")
                nc.vector.tensor_copy(cb[:, 0:D], ent[:])
                nc.vector.tensor_copy(cb[:, D:CROW], s8[:, H:2 * H])
                nc.scalar.dma_start(st68[i * P:(i + 1) * P, :], cb[:])
                if i % GT == GT - 1:
                    g = i // GT
                    nc.gpsimd.collective_compute(
                        "AllGather", mybir.AluOpType.bypass,
                        replica_groups=rg,
                        ins=[st68[g * GR:(g + 1) * GR, :].opt()],
                        outs=[comb[g * NCORES * GR:(g + 1) * NCORES * GR,
                                   :].opt()])

            def rel_sink(i, ent, s4):
                nc.vector.tensor_copy(sr_b[:, i * H:(i + 1) * H], s4[:, 0:H])

            table_pass(entloc_d, NTILES, wht_t, attht_t, 2 * HD, ent_sink)
            table_pass(relpad_d, 2, wr_t, attr_t, HD, rel_sink)

            import concourse.bass as _b

            # ---- power iterations ----
            for k in range(1, POW_ITER + 1):
                first = k == 1
                last = k == POW_ITER
                src = comb if first else z_full[(k - 2) % 2]
                rowlen = CROW if first else HD
                for i in range(NTILES):
                    zg = zgp.tile([P, CH * rowlen], bf16, tag="zg")
                    for j in range(CH):
                        nc.gpsimd.indirect_dma_start(
                            out=zg[:, j * rowlen:(j + 1) * rowlen],
                            out_offset=None, in_=src[:, :],
                            in_offset=_b.IndirectOffsetOnAxis(
                                ap=tti_t[:, i * CH + j:i * CH + j + 1],
                                axis=0))
                    # one-hot S^T for all chunks of this tile: [P, CH*P]
                    s6 = sop.tile([P, CH * P], bf16, tag="s")
                    nc.vector.tensor_tensor(
                        out=s6[:].rearrange("p (c n) -> p c n", c=CH),
                        in0=(hrel_t[:, i * CH:(i + 1) * CH]
                             .rearrange("p (c o) -> p c o", o=1)
                             .to_broadcast([P, CH, P])),
                        in1=(iota_f[:].rearrange("p (o n) -> p o n", o=1)
                             .to_broadcast([P, CH, P])),
                        op=OP.is_equal)
                    mw = MW if first else HD
                    if first:
                        # per-edge scores: s_h + s_r via one-hot matmuls
                        # (one-hot blocks loaded per TILE, not per chunk)
                        oh = ohp.tile([P, CH * P], bf16, tag="oh")
                        nc.sync.dma_start(
                            oh[:], h1h_d[:, i * CH * P:(i + 1) * CH * P])
                        r1a = ohp.tile([P, CH * P], bf16, tag="r1a")
                        nc.sync.dma_start(
                            r1a[:], r1h_d[0:P, i * CH * P:(i + 1) * CH * P])
                        r1b = ohp.tile([P, CH * P], bf16, tag="r1b")
                        nc.sync.dma_start(
                            r1b[:], r1h_d[P:2 * P,
                                          i * CH * P:(i + 1) * CH * P])
                        shr = ppC.tile([P, D], f32, tag="shr")
                        for j in range(CH):
                            nc.tensor.matmul(
                                shr[:, j * H:(j + 1) * H],
                                lhsT=oh[:, j * P:(j + 1) * P],
                                rhs=sh_all[:, i * H:(i + 1) * H],
                                start=True, stop=False)
                            nc.tensor.matmul(
                                shr[:, j * H:(j + 1) * H],
                                lhsT=r1a[:, j * P:(j + 1) * P],
                                rhs=sr_b[:, 0:H],
                                start=False, stop=False)
                            nc.tensor.matmul(
                                shr[:, j * H:(j + 1) * H],
                                lhsT=r1b[:, j * P:(j + 1) * P],
                                rhs=sr_b[:, H:2 * H],
                                start=False, stop=True)
                        # scores for whole tile: add s_t, leaky, exp
                        stf = sm.tile([P, CH * H], f32, tag="stf")
                        nc.vector.tensor_copy(
                            stf[:].rearrange("p (c h) -> p c h", c=CH),
                            zg[:].rearrange("p (c r) -> p c r", c=CH)
                            [:, :, D:CROW])
                        sc = sm.tile([P, CH * H], f32, tag="sc")
                        nc.vector.tensor_tensor(out=sc[:],
                                                in0=shr[:, 0:CH * H],
                                                in1=stf[:], op=OP.add)
                        sc2 = sm.tile([P, CH * H], f32, tag="sc2")
                        nc.vector.tensor_scalar_mul(sc2[:], sc[:], LEAKY)
                        nc.vector.tensor_tensor(out=sc[:], in0=sc[:],
                                                in1=sc2[:], op=OP.max)
                        nc.scalar.activation(
                            w_sb[:, i * CH * H:(i + 1) * CH * H], sc[:],
                            AF.Exp)
                    # messages for the whole tile
                    msg = msgp.tile([P, CH * MW], bf16, tag="msg")
                    wap = (w_sb[:, i * CH * H:(i + 1) * CH * H]
                           .rearrange("p (c h o) -> p c h o", c=CH, h=H)
                           .to_broadcast([P, CH, H, D]))
                    mview = (msg[:, 0:CH * mw]
                             .rearrange("p (c x) -> p c x", c=CH)
                             [:, :, 0:HD]
                             .rearrange("p c (h d) -> p c h d", h=H))
                    if first:
                        zs = (zg[:].rearrange("p (c r) -> p c r", c=CH)
                              [:, :, 0:D]
                              .rearrange("p c (o d) -> p c o d", o=1)
                              .to_broadcast([P, CH, H, D]))
                    else:
                        zs = zg[:].rearrange("p (c h d) -> p c h d",
                                             c=CH, h=H)
                    nc.vector.tensor_tensor(out=mview, in0=zs, in1=wap,
                                            op=OP.mult)
                    if first:
                        # denominator columns: msg[:, c*MW+HD : c*MW+MW] = w
                        nc.vector.tensor_copy(
                            (msg[:, 0:CH * MW]
                             .rearrange("p (c x) -> p c x", c=CH)
                             [:, :, HD:MW]),
                            (w_sb[:, i * CH * H:(i + 1) * CH * H]
                             .rearrange("p (c h) -> p c h", c=CH)))
                    ps = ppD.tile([P, MW], f32, tag="mm")
                    for j in range(CH):
                        nc.tensor.matmul(
                            ps[:, :mw], lhsT=s6[:, j * P:(j + 1) * P],
                            rhs=msg[:, j * mw:(j + 1) * mw],
                            start=(j == 0), stop=(j == CH - 1))
                    if first:
                        d1 = sm.tile([P, H], f32, tag="d1")
                        nc.vector.tensor_scalar_add(d1[:], ps[:, HD:MW], EPS)
                        d2 = sm.tile([P, H], f32, tag="d2")
                        nc.vector.reciprocal(d2[:], d1[:])
                        nc.vector.tensor_scalar_mul(
                            inv_sb[:, i * H:(i + 1) * H], d2[:], 1.0 - ALPHA)
                    # epilogue: zn = ps * inv + alpha * ent
                    zn = wk.tile([P, HD], bf16, tag="zn")
                    inv_b = (inv_sb[:, i * H:(i + 1) * H]
                             .rearrange("p (h o) -> p h o", o=1)
                             .to_broadcast([P, H, D]))
                    nc.vector.tensor_tensor(
                        out=zn[:].rearrange("p (h d) -> p h d", h=H),
                        in0=ps[:, 0:HD].rearrange("p (h d) -> p h d", h=H),
                        in1=inv_b, op=OP.mult)
                    ent_b = (aent[:, i * D:(i + 1) * D]
                             .rearrange("p (o d) -> p o d", o=1)
                             .to_broadcast([P, H, D]))
                    zn3 = zn[:].rearrange("p (h d) -> p h d", h=H)
                    nc.vector.tensor_tensor(out=zn3, in0=zn3, in1=ent_b,
                                            op=OP.add)
                    if not last:
                        nc.scalar.dma_start(z_shard[i * P:(i + 1) * P, :],
                                          zn[:])
                        if i % GT == GT - 1:
                            g = i // GT
                            nc.gpsimd.collective_compute(
                                "AllGather", mybir.AluOpType.bypass,
                                replica_groups=rg,
                                ins=[z_shard[g * GR:(g + 1) * GR, :].opt()],
                                outs=[z_full[(k - 1) % 2]
                                      [g * NCORES * GR:(g + 1) * NCORES * GR,
                                       :].opt()])
                    else:
                        po = ppC.tile([P, D], f32, tag="shr")
                        for b in range(HD // P):
                            tpp = ppB.tile([P, P], bf16, tag="tp")
                            nc.tensor.transpose(out=tpp[:],
                                                in_=zn[:, b * P:(b + 1) * P],
                                                identity=identb[:])
                            tps = wk.tile([P, P], bf16, tag="tps")
                            nc.scalar.activation(tps[:], tpp[:], AF.Copy)
                            nc.tensor.matmul(po[:, :], lhsT=tps[:],
                                             rhs=wo_t[:, b * D:(b + 1) * D],
                                             start=(b == 0),
                                             stop=(b == HD // P - 1))
                        ob = wk.tile([P, D], f32, tag="ob")
                        nc.vector.tensor_copy(ob[:], po[:, :])
                        nc.scalar.dma_start(out_d[i * P:(i + 1) * P, :], ob[:])
    nc.compile()
    return nc


def make_in_maps(cfg, cores, relpad_b, W_h, W_t, W_r, att_h, att_t,
                 att_r, W_o):
    import ml_dtypes

    bf16 = ml_dtypes.bfloat16

    def rep(att, n):
        a = np.concatenate([np.asarray(x, np.float32).reshape(1, HD)
                            for x in att], axis=1)
        return np.tile(a, (P, 1)).astype(bf16)

    wht = np.concatenate([np.asarray(W_h, np.float32),
                          np.asarray(W_t, np.float32)], axis=1).astype(bf16)
    wo = np.asarray(W_o, np.float32)  # [256, 64]
    wo_b = np.concatenate([wo[:P, :], wo[P:, :]], axis=1).astype(bf16)
    common = dict(
        relpad=np.ascontiguousarray(relpad_b),
        wht=np.ascontiguousarray(wht),
        wr=np.ascontiguousarray(np.asarray(W_r, np.float32).astype(bf16)),
        wo=np.ascontiguousarray(wo_b),
        attht=np.ascontiguousarray(rep([att_h, att_t], 2)),
        attr=np.ascontiguousarray(rep([att_r], 1)),
    )
    in_maps = []
    for c in range(NCORES):
        m = dict(common)
        m["entloc"] = np.ascontiguousarray(cores[c]["entloc"])
        m["hrel"] = np.ascontiguousarray(cores[c]["hrel"])
        m["tti"] = np.ascontiguousarray(cores[c]["tti"])
        m["h1h"] = np.ascontiguousarray(cores[c]["h1h"])
        m["r1h"] = np.ascontiguousarray(cores[c]["r1h"])
        in_maps.append(m)
    return in_maps


_CACHE = {}


def kernel(entity_embed, relation_embed, W_h, W_t, W_r, att_h, att_t, att_r,
           W_o, edge_index, edge_type):
    from concourse.bass_utils import run_bass_kernel_spmd

    cfg = Cfg(640)
    cfg, cores, relpad_b, newpos = host_prep(
        cfg, entity_embed, relation_embed, edge_index, edge_type)
    in_maps = make_in_maps(cfg, cores, relpad_b, W_h, W_t, W_r,
                           att_h, att_t, att_r, W_o)
    key = cfg.l_cap
    if key not in _CACHE:
        _CACHE[key] = build_program(cfg)
    nc = _CACHE[key]
    res = run_bass_kernel_spmd(nc, in_maps, core_ids=list(range(NCORES)))
    full = np.concatenate(
        [res.results[c]["out"] for c in range(NCORES)], axis=0)
    return full[newpos].astype(np.float32)
